# revision 2
# baseline (speedup 1.0000x reference)
"""Trainium2 Bass kernel for nn_GAT_n2v_mean (3-layer edge-featured GAT +
mean-pool + MLP), v2.

Strategy: edges partitioned by dst owner across 8 cores (6250 nodes each,
127-node blocks + trash slot). Per-edge src features come from batched
int16-indexed dma_gathers (<=1024 idx/instr, trailing -1 skipped) out of
bf16 node-feature tables with 256B-aligned row strides. Attention-logit
source/dst terms ride the tables as extra matmul columns (W_ext); the dst
term is expanded per edge via a host-baked transposed one-hot (snsT) and
small PE matmuls. Layer 1's table is computed locally on every device (x is
replicated), so only layers 2/3 AllGather their tables. Segment softmax uses
a per-device shift (max al_s + max al_d over produced rows) which is exact
for dst-owned segments - no AllReduce needed. Scatter-adds are one-hot
matmuls accumulating in PSUM; everything data-sized is bf16, accumulation
fp32.
"""

import numpy as np
import ml_dtypes

BFL = ml_dtypes.bfloat16

# ---------------------------------------------------------------- host config
N, E, G, D = 50000, 800000, 64, 8
NPD = N // D
BLK = 127
NB = (NPD + BLK - 1) // BLK       # 50
R = NB * 128                       # 6400 padded local rows
QROWS = 4 * R                      # 25600 rows per quad table slice
EPS = 1e-5
BNC = float(1.0 / np.sqrt(1.0 + EPS))
DIMS = [(32, 4, 64), (256, 4, 128), (512, 4, 64)]
HCs = [256, 512, 256]
LIVE = [264, 520, 264]             # HC + 8 live row elems
STRIDE = [384, 640, 384]           # padded table row stride (x2B % 256 == 0)
GCAP = 1024                        # max idxs per dma_gather instruction

_CACHE = {}


def _wrap_idx(ids, ncols, pad=-1):
    """int16 idx layout: value i at [16k + i%16, i//16] for k in 0..7."""
    a = np.full((128, ncols), pad, np.int16)
    n = len(ids)
    cols = (n + 15) // 16
    block = np.full(16 * cols, pad, np.int16)
    block[:n] = ids
    block = block.reshape(cols, 16).T          # [16, cols]
    for k in range(8):
        a[16 * k:16 * k + 16, :cols] = block
    return a


def _prep(inputs, simpad=True):
    x = np.asarray(inputs["x"], np.float32)
    ef = np.asarray(inputs["edge_feature"], np.float32)
    src_g = np.asarray(inputs["edge_index"][0], np.int64)
    dst_g = np.asarray(inputs["edge_index"][1], np.int64)
    batch = np.asarray(inputs["batch"], np.int64)

    def gr(n):   # global padded row id
        return (n // NPD) * R + (n % NPD)

    # ---- per-device edge layout
    per_dev = []
    max_cnt = 0
    for d in range(D):
        m = (dst_g // NPD) == d
        s, t, f = src_g[m], dst_g[m], ef[m]
        loc = t - d * NPD
        b = loc // BLK
        rel = loc % BLK
        srow = gr(s)
        q = srow // QROWS
        order = np.lexsort((np.arange(len(s)), q, b))
        s, f, b, rel, srow, q = (a[order] for a in (s, f, b, rel, srow, q))
        cnts = np.zeros((NB, 2), np.int64)
        np.add.at(cnts, (b, q), 1)
        max_cnt = max(max_cnt, int(cnts.max()))
        per_dev.append((f, b, rel, srow, q, cnts))

    cA = (min(max_cnt, GCAP) + 127) // 128
    rest = max(max_cnt - GCAP, 16)
    cB = (rest + 127) // 128 if max_cnt > GCAP else 1
    T = 2 * (cA + cB)
    cfg = (T, cA, cB)

    segs = [(0, 0, cA), (0, cA, cB), (1, cA + cB, cA), (1, 2 * cA + cB, cB)]
    # (quad, chunk base, n chunks); segment A first min(cnt, cA*128) edges

    in_maps = []
    for d in range(D):
        f, b, rel, srow, q, cnts = per_dev[d]
        off = np.zeros((NB, 2), np.int64)
        # start offset of (block, quad) runs in the sorted edge arrays
        flat = np.concatenate([[0], np.cumsum(cnts.reshape(-1))])[:-1]
        off = flat.reshape(NB, 2)

        rels = np.full((NB, T * 128), 127, np.int64)
        pad = 0 if simpad else -1
        idx16 = np.full((NB, 128, T * 8), pad, np.int16)
        eatt = np.zeros((NB, 6, T * 128), np.float32)
        eart = np.zeros((NB, 128, T * 8), np.float32)

        for blk in range(NB):
            for (qq, cbase, nch) in segs:
                e0 = off[blk, qq]
                cnt = cnts[blk, qq]
                is_a = cbase in (0, cA + cB)
                if is_a:
                    k = min(cnt, cA * 128)
                    el = np.arange(e0, e0 + k)
                    ids = (srow[el] % QROWS).astype(np.int16)
                    rl = rel[el]
                    fe = f[el]
                else:
                    k0 = min(cnt, cA * 128)
                    k = cnt - k0
                    if k > 0:
                        el = np.arange(e0 + k0, e0 + cnt)
                        ids = (srow[el] % QROWS).astype(np.int16)
                        rl = rel[el]
                        fe = f[el]
                    else:
                        k = 16
                        ids = np.zeros(16, np.int16)
                        rl = np.full(16, 127, np.int64)
                        fe = np.zeros((16, 6), np.float32)
                assert k <= nch * 128, (d, blk, qq, k, nch)
                ii = np.arange(k)
                sl = (cbase + ii // 128) * 128 + ii % 128
                rels[blk, sl] = rl
                eatt[blk, :, sl] = fe
                tt = sl // 128
                pp = sl % 128
                for j in range(6):
                    eart[blk, pp, tt * 8 + j] = fe[:, j]
                eart[blk, pp, tt * 8 + 6] = 1.0
                idx16[blk, :, cbase * 8:(cbase + nch) * 8] = _wrap_idx(
                    ids, nch * 8, pad)

        # snsT[n, t*128+p] = 1{rels[t*128+p] == n}
        snsT = (rels[:, None, :] == np.arange(128)[None, :, None])
        snsT = snsT.astype(BFL)                       # [NB, 128, T*128]

        # rel per (block, slot) -> [128, T] tile layout: rel[p, t] = slot t*128+p
        relb = rels.reshape(NB, T, 128).transpose(0, 2, 1).astype(BFL)

        bcol = np.full((NB, 128), -1.0, np.float32)
        for blk in range(NB):
            lo = blk * BLK
            nreal = min(BLK, NPD - lo)
            if nreal > 0:
                bcol[blk, :nreal] = batch[d * NPD + lo: d * NPD + lo + nreal]

        xT_full = np.zeros((32, D * R), np.float32)
        for dd in range(D):
            xT_full[:, dd * R:dd * R + NPD] = x[dd * NPD:(dd + 1) * NPD].T
        xT_loc = np.zeros((32, R), np.float32)
        xT_loc[:, :NPD] = x[d * NPD:(d + 1) * NPD].T

        im = {
            "relb": relb.reshape(NB * 128, T).copy(),
            "gidx": idx16.reshape(NB * 128, T * 8).copy(),
            "snsT": snsT.reshape(NB * 128, T * 128).copy(),
            "eaT": np.ascontiguousarray(eatt.transpose(1, 0, 2)
                                        .reshape(6, NB * T * 128)).astype(BFL),
            "eart": eart.reshape(NB * 128, T * 8).astype(BFL),
            "bcol": bcol.reshape(NB * 128, 1).astype(BFL),
            "xT_full": xT_full.astype(BFL),
            "xT_loc": xT_loc.astype(BFL),
            "iota128": np.broadcast_to(np.arange(128, dtype=np.float32),
                                       (128, 128)).astype(BFL).copy(),
            "iota64": np.broadcast_to(np.arange(64, dtype=np.float32),
                                      (128, 64)).astype(BFL).copy(),
            "identb": np.eye(128, dtype=np.float32).astype(BFL),
            "identf": np.eye(128, dtype=np.float32),
            "onescol": np.ones((128, 1), np.float32).astype(BFL),
        }

        Aecat = np.zeros((6, 12), np.float32)
        for li, (fin, H, C) in enumerate(DIMS, 1):
            HC = H * C
            W = np.asarray(inputs[f"W{li}"], np.float32)
            a_s = np.asarray(inputs[f"as{li}"], np.float32)
            a_d = np.asarray(inputs[f"ad{li}"], np.float32)
            a_e = np.asarray(inputs[f"ae{li}"], np.float32)
            We = np.asarray(inputs[f"We{li}"], np.float32)
            Was = np.stack([W[:, h * C:(h + 1) * C] @ a_s[h]
                            for h in range(H)], axis=1)
            Wad = np.stack([W[:, h * C:(h + 1) * C] @ a_d[h]
                            for h in range(H)], axis=1)
            Wext = np.concatenate([W, Was, Wad], axis=1)   # [fin, HC+8]
            im[f"Wext{li}"] = Wext.astype(BFL)
            Aecat[:, (li - 1) * 4: li * 4] = np.stack(
                [We[:, h * C:(h + 1) * C] @ a_e[h] for h in range(H)], axis=1)
            g = np.asarray(inputs[f"g{li}"], np.float32)
            bb = np.asarray(inputs[f"bb{li}"], np.float32)
            bl = np.asarray(inputs[f"b{li}"], np.float32)
            ghat = g * BNC
            b2 = ghat * bl + bb
            im[f"ghat{li}"] = np.broadcast_to(ghat, (128, HC)).astype(BFL).copy()
            im[f"b2{li}"] = np.broadcast_to(b2, (128, HC)).astype(BFL).copy()
        im["Aecat"] = Aecat.astype(BFL)
        im["Wf1"] = np.asarray(inputs["Wf1"], np.float32)
        im["Wf2"] = np.asarray(inputs["Wf2"], np.float32)
        gf = np.asarray(inputs["gf"], np.float32) * BNC
        b2f = gf * np.asarray(inputs["bf1"], np.float32) + \
            np.asarray(inputs["bbf"], np.float32)
        im["gfr"] = np.broadcast_to(gf, (64, 32)).copy()
        im["b2fr"] = np.broadcast_to(b2f, (64, 32)).copy()
        im["bf2r"] = np.broadcast_to(np.asarray(inputs["bf2"], np.float32),
                                     (64, 2)).copy()
        in_maps.append(im)
    return in_maps, cfg


# ---------------------------------------------------------------- device prog
def _build(cfg, unroll=False, dbg=False):
    import concourse.bass as bass
    import concourse.bacc as bacc
    import concourse.mybir as mybir
    import concourse.tile as tile
    from contextlib import ExitStack

    T, cA, cB = cfg
    f32 = mybir.dt.float32
    bf16 = mybir.dt.bfloat16
    i16 = mybir.dt.int16
    AO = mybir.AluOpType
    AF = mybir.ActivationFunctionType
    AX = mybir.AxisListType
    RG = [list(range(D))]
    segs = [(0, 0, cA), (0, cA, cB), (1, cA + cB, cA), (1, 2 * cA + cB, cB)]

    nc = bacc.Bacc(None, target_bir_lowering=False, debug=True)

    inp = {}

    def di(name, shape, dt=bf16):
        inp[name] = nc.declare_dram_parameter(name, list(shape), dt,
                                              isOutput=False)
        return inp[name]

    di("relb", (NB * 128, T))
    di("gidx", (NB * 128, T * 8), i16)
    di("snsT", (NB * 128, T * 128))
    di("eaT", (6, NB * T * 128))
    di("eart", (NB * 128, T * 8))
    di("bcol", (NB * 128, 1))
    di("xT_full", (32, D * R))
    di("xT_loc", (32, R))
    di("iota128", (128, 128)); di("iota64", (128, 64))
    di("identb", (128, 128)); di("identf", (128, 128), f32)
    di("onescol", (128, 1))
    di("Aecat", (6, 12))
    for li, (fin, H, C) in enumerate(DIMS, 1):
        HC = H * C
        di(f"Wext{li}", (fin, HC + 8))
        di(f"ghat{li}", (128, HC)); di(f"b2{li}", (128, HC))
    di("Wf1", (256, 32), f32); di("Wf2", (32, 2), f32)
    di("gfr", (64, 32), f32); di("b2fr", (64, 32), f32)
    di("bf2r", (64, 2), f32)
    out_d = nc.declare_dram_parameter("out", [64, 2], f32, isOutput=True)
    dbg_d = {}
    if dbg:
        for nm, shp in [("dxf1", (128, 384)), ("dxe2", (128, 640)),
                        ("dxe3", (128, 384)), ("dale", (128, 12 * T)),
                        ("demean", (8, 128)), ("dpool", (64, 257)),
                        ("dmh", (128, 12))]:
            dbg_d[nm] = nc.declare_dram_parameter(nm, list(shp), f32,
                                                  isOutput=True)

    # internal DRAM
    xf = [nc.dram_tensor(f"xf{l}", [D * R, STRIDE[l - 1]], bf16,
                         **({"addr_space": "Shared"} if l > 1 else {}))
          for l in (1, 2, 3)]
    xe = [nc.dram_tensor(f"xe{l}", [R, STRIDE[l - 1]], bf16) for l in (1, 2, 3)]
    ale_d = nc.dram_tensor("ale", [NB * 128, 12 * T], bf16)
    emeanT_d = nc.dram_tensor("emeanT", [8, NB * 128], bf16)
    pool_i = nc.dram_tensor("pool_i", [64, 257], f32)
    pool_o = nc.dram_tensor("pool_o", [64, 257], f32, addr_space="Shared")

    with ExitStack() as ctx:
        tc = ctx.enter_context(tile.TileContext(nc))

        def loop(n, body):
            if unroll:
                for i in range(n):
                    body(i)
            else:
                with tc.For_i(0, n, 1) as i:
                    body(i)

        consts = ctx.enter_context(tc.tile_pool(name="consts", bufs=1))
        lay = ctx.enter_context(tc.tile_pool(name="lay", bufs=1))
        glob = ctx.enter_context(tc.tile_pool(name="glob", bufs=1))
        sb = ctx.enter_context(tc.tile_pool(name="sb", bufs=2))
        sbg = ctx.enter_context(tc.tile_pool(name="sbg", bufs=2))
        sbv = ctx.enter_context(tc.tile_pool(name="sbv", bufs=3))
        psb = ctx.enter_context(tc.tile_pool(name="psb", bufs=2, space="PSUM"))
        pss = ctx.enter_context(tc.tile_pool(name="pss", bufs=2, space="PSUM"))
        pst = ctx.enter_context(tc.tile_pool(name="pst", bufs=3, space="PSUM"))

        io128 = consts.tile([128, 128], bf16)
        nc.sync.dma_start(out=io128[:], in_=inp["iota128"][:])
        io64 = consts.tile([128, 64], bf16)
        nc.sync.dma_start(out=io64[:], in_=inp["iota64"][:])
        identb = consts.tile([128, 128], bf16)
        nc.sync.dma_start(out=identb[:], in_=inp["identb"][:])
        identf = consts.tile([128, 128], f32)
        nc.sync.dma_start(out=identf[:], in_=inp["identf"][:])
        onescol = consts.tile([128, 1], bf16)
        nc.sync.dma_start(out=onescol[:], in_=inp["onescol"][:])
        onesrow = consts.tile([1, 128], f32)
        nc.any.memset(onesrow[:], 1.0)
        Aecat = consts.tile([6, 12], bf16)
        nc.sync.dma_start(out=Aecat[:], in_=inp["Aecat"][:])
        w1 = consts.tile([32, 264], bf16)
        nc.sync.dma_start(out=w1[:], in_=inp["Wext1"][:])

        mx = [glob.tile([128, 8], f32, tag=f"mx{l}", name=f"mx{l}")
              for l in (1, 2, 3)]
        for t_ in mx:
            nc.any.memset(t_[:], -3e38)
        mhat = [glob.tile([128, 4], f32, tag=f"mh{l}", name=f"mh{l}")
                for l in (1, 2, 3)]
        pool_sb = glob.tile([64, 257], f32)
        nc.any.memset(pool_sb[:], 0.0)

        # ---------------- phase 1: xf1 / xe1 local projection ----------------
        def proj_body(rt, src, dst, track_mx):
            st = rt * 128
            ht = sbv.tile([32, 128], bf16, tag="pht")
            nc.sync.dma_start(out=ht[:], in_=src[:, bass.ds(st, 128)])
            pxs = pst.tile([128, 264], f32, tag="ps3")
            nc.tensor.matmul(pxs[:], ht[:], w1[:], start=True, stop=True)
            xs = sbv.tile([128, 264], bf16, tag="pxs")
            nc.scalar.activation(out=xs[:], in_=pxs[:], func=AF.Copy)
            if track_mx:
                nc.vector.tensor_tensor(out=mx[0][:], in0=mx[0][:],
                                        in1=pxs[:, 256:264], op=AO.max)
            nc.sync.dma_start(out=dst[bass.ds(st, 128), 0:264], in_=xs[:])

        loop(D * R // 128,
             lambda rt: proj_body(rt, inp["xT_full"], xf[0], True))
        loop(R // 128, lambda rt: proj_body(rt, inp["xT_loc"], xe[0], False))

        # ---------------- phase 2: precompute emean + al_e ----------------
        def pre_body(i):
            st = i * 128
            rel = sb.tile([128, T], bf16, tag="prel")
            nc.sync.dma_start(out=rel[:], in_=inp["relb"][bass.ds(st, 128), :])
            ear = sb.tile([128, T * 8], bf16, tag="pear")
            nc.sync.dma_start(out=ear[:], in_=inp["eart"][bass.ds(st, 128), :])
            eatt = sb.tile([6, T * 128], bf16, tag="peatt")
            nc.scalar.dma_start(out=eatt[:],
                                in_=inp["eaT"][:, bass.ts(i, T * 128)])
            sall = sbg.tile([128, T * 128], bf16, tag="sall")
            nc.vector.tensor_tensor(
                out=sall[:].rearrange("p (t n) -> p t n", t=T),
                in0=rel[:].unsqueeze(2).to_broadcast([128, T, 128]),
                in1=io128[:].unsqueeze(1).to_broadcast([128, T, 128]),
                op=AO.is_equal)
            acc = pss.tile([128, 8], f32, tag="sm")
            alp = psb.tile([128, 12 * T], f32, tag="big")
            for t in range(T):
                nc.tensor.matmul(acc[:], sall[:, t * 128:(t + 1) * 128],
                                 ear[:, t * 8:(t + 1) * 8],
                                 start=(t == 0), stop=(t == T - 1))
                nc.tensor.matmul(alp[:, t * 12:(t + 1) * 12],
                                 eatt[:, t * 128:(t + 1) * 128],
                                 Aecat[:], start=True, stop=True)
            ale3 = sb.tile([128, 12 * T], bf16, tag="ale3")
            nc.vector.tensor_copy(
                out=ale3[:].rearrange("p (l t h) -> p l t h", l=3, t=T),
                in_=alp[:].rearrange("p (t l h) -> p l t h", t=T, l=3))
            nc.scalar.dma_start(out=ale_d[bass.ds(st, 128), :], in_=ale3[:])
            degc = sb.tile([128, 1], f32, tag="degc")
            nc.vector.tensor_scalar(out=degc[:], in0=acc[:, 6:7], scalar1=1.0,
                                    scalar2=None, op0=AO.max)
            nc.vector.reciprocal(out=degc[:], in_=degc[:])
            em = sb.tile([128, 8], f32, tag="em")
            nc.vector.tensor_scalar(out=em[:], in0=acc[:], scalar1=degc[:],
                                    scalar2=None, op0=AO.mult)
            emt_ps = pst.tile([8, 128], f32, tag="ps3")
            nc.tensor.transpose(emt_ps[:], em[:], identf[:])
            emt = sb.tile([8, 128], bf16, tag="emts")
            nc.vector.tensor_copy(out=emt[:], in_=emt_ps[:])
            nc.sync.dma_start(out=emeanT_d[:, bass.ds(st, 128)], in_=emt[:])

        loop(NB, pre_body)

        # mhat1 from mx[0]
        def build_mhat(l):
            mx_ps = pst.tile([8, 128], f32, tag="ps3")
            nc.tensor.transpose(mx_ps[:], mx[l - 1][:], identf[:])
            mx_sb = sb.tile([8, 128], f32, tag="mxsb")
            nc.vector.tensor_copy(out=mx_sb[:], in_=mx_ps[:])
            t32 = sb.tile([32, 32], f32, tag="t32")
            nc.any.memset(t32[:], -3e38)
            nc.vector.tensor_reduce(out=t32[0:8, 0:1], in_=mx_sb[:],
                                    axis=AX.X, op=AO.max)
            v32 = sb.tile([32, 32], f32, tag="v32")
            nc.vector.transpose(out=v32[:], in_=t32[:])
            mh1 = sb.tile([1, 4], f32, tag="mh1")
            nc.vector.tensor_tensor(out=mh1[:], in0=v32[0:1, 0:4],
                                    in1=v32[0:1, 4:8], op=AO.add)
            mh_ps = pst.tile([128, 4], f32, tag="ps3")
            nc.tensor.matmul(mh_ps[:], onesrow[:], mh1[:], start=True,
                             stop=True)
            nc.vector.tensor_copy(out=mhat[l - 1][:], in_=mh_ps[:])

        build_mhat(1)

        # ---------------- phase 3: attention layers ----------------
        for li, (fin, H, C) in enumerate(DIMS, 1):
            HC = H * C
            S = STRIDE[li - 1]
            LV = LIVE[li - 1]
            xf_l = xf[li - 1]
            if li < 3:
                HCn = HCs[li]
                Sn = STRIDE[li]
                nk = HC // 128
                wsb = lay.tile([128, nk * (HCn + 8)], bf16, tag="wsb")
                for k in range(nk):
                    nc.sync.dma_start(
                        out=wsb[:, k * (HCn + 8):(k + 1) * (HCn + 8)],
                        in_=inp[f"Wext{li + 1}"][k * 128:(k + 1) * 128, :])
            ghat = lay.tile([128, HC], bf16, tag="ghat")
            nc.sync.dma_start(out=ghat[:], in_=inp[f"ghat{li}"][:])
            b2 = lay.tile([128, HC], bf16, tag="b2")
            nc.sync.dma_start(out=b2[:], in_=inp[f"b2{li}"][:])

            def att_body(i, li=li, HC=HC, S=S, LV=LV, xf_l=xf_l, H=H, C=C):
                st = i * 128
                stb = i * BLK
                rel = sb.tile([128, T], bf16, tag="arel")
                nc.sync.dma_start(out=rel[:],
                                  in_=inp["relb"][bass.ds(st, 128), :])
                gix = sb.tile([128, T * 8], i16, tag="gix")
                nc.sync.dma_start(out=gix[:],
                                  in_=inp["gidx"][bass.ds(st, 128), :])
                snsT = sbg.tile([128, T * 128], bf16, tag="snsT")
                nc.scalar.dma_start(out=snsT[:],
                                    in_=inp["snsT"][bass.ds(st, 128), :])
                xsl = sb.tile([128, LV], bf16, tag="xsl")
                nc.sync.dma_start(out=xsl[:],
                                  in_=xe[li - 1][bass.ds(stb, 128), 0:LV])
                alet = sb.tile([128, 4 * T], bf16, tag="alet")
                nc.sync.dma_start(
                    out=alet[:],
                    in_=ale_d[bass.ds(st, 128),
                              (li - 1) * 4 * T: li * 4 * T])
                emt = sb.tile([8, 128], bf16, tag="emt")
                nc.scalar.dma_start(out=emt[:],
                                    in_=emeanT_d[:, bass.ds(st, 128)])
                gat = sbg.tile([128, T * S], bf16, tag="gat")
                for (qq, cbase, nch) in segs:
                    nc.gpsimd.dma_gather(
                        gat[:, cbase * S:(cbase + nch) * S].rearrange(
                            "p (c w) -> p c w", c=nch),
                        xf_l[qq * QROWS:(qq + 1) * QROWS, :],
                        gix[:, cbase * 8:(cbase + nch) * 8],
                        nch * 128, nch * 128, S, elem_step=S)
                sall = sbg.tile([128, T * 128], bf16, tag="sall")
                nc.vector.tensor_tensor(
                    out=sall[:].rearrange("p (t n) -> p t n", t=T),
                    in0=rel[:].unsqueeze(2).to_broadcast([128, T, 128]),
                    in1=io128[:].unsqueeze(1).to_broadcast([128, T, 128]),
                    op=AO.is_equal)
                atp = pss.tile([128, 4 * T], f32, tag="sm")
                for t in range(T):
                    nc.tensor.matmul(atp[:, t * 4:(t + 1) * 4],
                                     snsT[:, t * 128:(t + 1) * 128],
                                     xsl[:, HC + 4:HC + 8],
                                     start=True, stop=True)
                wall = sb.tile([128, 4 * T], f32, tag="wall")
                nc.vector.tensor_tensor(
                    out=wall[:],
                    in0=gat[:].rearrange("p (t s) -> p t s", t=T)[
                        :, :, HC:HC + 4],
                    in1=atp[:], op=AO.add)
                nc.vector.tensor_tensor(out=wall[:], in0=wall[:],
                                        in1=alet[:], op=AO.add)
                nc.vector.scalar_tensor_tensor(
                    out=wall[:], in0=wall[:], scalar=0.2, in1=wall[:],
                    op0=AO.mult, op1=AO.max)
                nc.vector.tensor_tensor(
                    out=wall[:].rearrange("p (t h) -> p t h", t=T),
                    in0=wall[:].rearrange("p (t h) -> p t h", t=T),
                    in1=mhat[li - 1][:].unsqueeze(1).to_broadcast([128, T, 4]),
                    op=AO.subtract)
                walle = sb.tile([128, 4 * T], bf16, tag="walle")
                nc.scalar.activation(out=walle[:], in_=wall[:], func=AF.Exp)
                nps = psb.tile([128, HC], f32, tag="big")
                dps = pss.tile([128, 4], f32, tag="sm")
                for t in range(T):
                    val = sbv.tile([128, HC], bf16, tag="val")
                    nc.vector.tensor_tensor(
                        out=val[:].rearrange("p (h c) -> p h c", h=H),
                        in0=gat[:, t * S:t * S + HC].rearrange(
                            "p (h c) -> p h c", h=H),
                        in1=walle[:, t * 4:(t + 1) * 4].unsqueeze(2)
                        .to_broadcast([128, 4, C]),
                        op=AO.mult)
                    nc.tensor.matmul(nps[:], sall[:, t * 128:(t + 1) * 128],
                                     val[:], start=(t == 0), stop=(t == T - 1))
                    nc.tensor.matmul(dps[:], sall[:, t * 128:(t + 1) * 128],
                                     walle[:, t * 4:(t + 1) * 4],
                                     start=(t == 0), stop=(t == T - 1))
                # epilogue: self-loop + normalize + BN + ELU
                aesp = pst.tile([128, 4], f32, tag="ps3")
                nc.tensor.matmul(aesp[:], emt[0:6, :],
                                 Aecat[:, (li - 1) * 4:li * 4],
                                 start=True, stop=True)
                als = sb.tile([128, 4], f32, tag="als")
                nc.vector.tensor_tensor(out=als[:], in0=xsl[:, HC:HC + 4],
                                        in1=xsl[:, HC + 4:HC + 8], op=AO.add)
                nc.vector.tensor_tensor(out=als[:], in0=als[:], in1=aesp[:],
                                        op=AO.add)
                nc.vector.scalar_tensor_tensor(
                    out=als[:], in0=als[:], scalar=0.2, in1=als[:],
                    op0=AO.mult, op1=AO.max)
                nc.vector.tensor_tensor(out=als[:], in0=als[:],
                                        in1=mhat[li - 1][:], op=AO.subtract)
                alse = sb.tile([128, 4], f32, tag="alse")
                nc.scalar.activation(out=alse[:], in_=als[:], func=AF.Exp)
                den = sb.tile([128, 4], f32, tag="den")
                nc.vector.tensor_tensor(out=den[:], in0=dps[:], in1=alse[:],
                                        op=AO.add)
                nc.vector.reciprocal(out=den[:], in_=den[:])
                hh = sb.tile([128, HC], bf16, tag="hh")
                for h in range(H):
                    hsl = hh[:, h * C:(h + 1) * C]
                    nc.vector.scalar_tensor_tensor(
                        out=hsl, in0=xsl[:, h * C:(h + 1) * C],
                        scalar=alse[:, h:h + 1],
                        in1=nps[:, h * C:(h + 1) * C],
                        op0=AO.mult, op1=AO.add)
                    nc.vector.tensor_scalar(
                        out=hsl, in0=hsl, scalar1=den[:, h:h + 1],
                        scalar2=None, op0=AO.mult)
                nc.vector.tensor_tensor(out=hh[:], in0=hh[:], in1=ghat[:],
                                        op=AO.mult)
                nc.vector.tensor_tensor(out=hh[:], in0=hh[:], in1=b2[:],
                                        op=AO.add)
                zn = sb.tile([128, HC], bf16, tag="zn")
                nc.vector.tensor_scalar(out=zn[:], in0=hh[:], scalar1=0.0,
                                        scalar2=None, op0=AO.min)
                nc.scalar.activation(out=zn[:], in_=zn[:], func=AF.Exp)
                rl = sb.tile([128, HC], bf16, tag="rl")
                nc.scalar.activation(out=rl[:], in_=hh[:], func=AF.Relu)
                nc.vector.scalar_tensor_tensor(
                    out=hh[:], in0=zn[:], scalar=-1.0, in1=rl[:],
                    op0=AO.add, op1=AO.add)
                if li < 3:
                    HCn = HCs[li]
                    nk = HC // 128
                    pxa = psb.tile([128, min(HCn, 512)], f32, tag="big")
                    pxb = pss.tile([128, 8], f32, tag="sm")
                    for k in range(nk):
                        trp = pst.tile([128, 128], bf16, tag="ps3")
                        nc.tensor.transpose(trp[:],
                                            hh[:, k * 128:(k + 1) * 128],
                                            identb[:])
                        htc = sbv.tile([128, 128], bf16, tag="htc")
                        nc.vector.tensor_copy(out=htc[:], in_=trp[:])
                        wk = wsb[:, k * (HCn + 8):(k + 1) * (HCn + 8)]
                        nc.tensor.matmul(pxa[:], htc[:], wk[:, 0:HCn],
                                         start=(k == 0), stop=(k == nk - 1))
                        nc.tensor.matmul(pxb[:], htc[:], wk[:, HCn:HCn + 8],
                                         start=(k == 0), stop=(k == nk - 1))
                    xs = sb.tile([128, HCn + 8], bf16, tag="xsout")
                    nc.scalar.activation(out=xs[:, 0:HCn], in_=pxa[:],
                                         func=AF.Copy)
                    nc.vector.tensor_copy(out=xs[:, HCn:HCn + 8], in_=pxb[:])
                    nc.vector.tensor_tensor(out=mx[li][0:127, :],
                                            in0=mx[li][0:127, :],
                                            in1=xs[0:127, HCn:HCn + 8],
                                            op=AO.max)
                    nc.sync.dma_start(
                        out=xe[li][bass.ds(stb, 128), 0:HCn + 8],
                        in_=xs[:])
                else:
                    bc = sb.tile([128, 1], bf16, tag="bc")
                    nc.sync.dma_start(out=bc[:],
                                      in_=inp["bcol"][bass.ds(st, 128), :])
                    bt = sb.tile([128, 64], bf16, tag="bt")
                    nc.vector.tensor_tensor(out=bt[:],
                                            in0=bc[:].to_broadcast([128, 64]),
                                            in1=io64[:], op=AO.is_equal)
                    pps = pst.tile([64, 257], f32, tag="ps3")
                    nc.tensor.matmul(pps[:, 0:HC], bt[:], hh[:],
                                     start=True, stop=True)
                    nc.tensor.matmul(pps[:, 256:257], bt[:], onescol[:],
                                     start=True, stop=True)
                    nc.vector.tensor_tensor(out=pool_sb[:], in0=pool_sb[:],
                                            in1=pps[:], op=AO.add)

            loop(NB, att_body)
            if li < 3:
                build_mhat(li + 1)
                nc.gpsimd.collective_compute(
                    "AllGather", AO.bypass, replica_groups=RG,
                    ins=[xe[li][:]], outs=[xf[li][:]])

        if dbg:
            def dump(nm, src, shape, dt=bf16):
                tl = sb.tile(list(shape), dt, tag="dbgl", name="dbgtl")
                nc.sync.dma_start(out=tl[:], in_=src)
                tf = sb.tile(list(shape), f32, tag="dbgf", name="dbgtf")
                nc.vector.tensor_copy(out=tf[:], in_=tl[:])
                nc.sync.dma_start(out=dbg_d[nm][:], in_=tf[:])
            dump("dxf1", xf[0][0:128, :], (128, 384))
            dump("dxe2", xe[1][0:128, :], (128, 640))
            dump("dxe3", xe[2][0:128, :], (128, 384))
            dump("dale", ale_d[0:128, :], (128, 12 * T))
            dump("demean", emeanT_d[:, 0:128], (8, 128))
            nc.sync.dma_start(out=dbg_d["dpool"][:], in_=pool_sb[:])
            mhcat = sb.tile([128, 12], f32, tag="mhcat")
            for l_ in range(3):
                nc.vector.tensor_copy(out=mhcat[:, l_ * 4:(l_ + 1) * 4],
                                      in_=mhat[l_][:])
            nc.sync.dma_start(out=dbg_d["dmh"][:], in_=mhcat[:])

        # ---------------- final MLP ----------------
        nc.sync.dma_start(out=pool_i[:], in_=pool_sb[:])
        nc.gpsimd.collective_compute("AllReduce", AO.add, replica_groups=RG,
                                     ins=[pool_i[:]], outs=[pool_o[:]])
        pool2 = sb.tile([64, 257], f32, tag="pool2")
        nc.sync.dma_start(out=pool2[:], in_=pool_o[:])
        cnt = sb.tile([64, 1], f32, tag="cnt")
        nc.vector.tensor_scalar(out=cnt[:], in0=pool2[:, 256:257], scalar1=1.0,
                                scalar2=None, op0=AO.max)
        nc.vector.reciprocal(out=cnt[:], in_=cnt[:])
        nc.vector.tensor_scalar(out=pool2[:, 0:256], in0=pool2[:, 0:256],
                                scalar1=cnt[:], scalar2=None, op0=AO.mult)
        pts = sb.tile([128, 128], f32, tag="pts")
        for ch in range(2):
            ptp = pst.tile([128, 64], f32, tag="ps3")
            nc.tensor.transpose(ptp[:], pool2[:, ch * 128:(ch + 1) * 128],
                                identf[0:64, 0:64])
            nc.vector.tensor_copy(out=pts[:, ch * 64:(ch + 1) * 64],
                                  in_=ptp[:])
        wf1 = sb.tile([128, 64], f32, tag="wf1")
        for ch in range(2):
            nc.sync.dma_start(out=wf1[:, ch * 32:(ch + 1) * 32],
                              in_=inp["Wf1"][ch * 128:(ch + 1) * 128, :])
        z1p = pst.tile([64, 32], f32, tag="ps3")
        for ch in range(2):
            nc.tensor.matmul(z1p[:], pts[:, ch * 64:(ch + 1) * 64],
                             wf1[:, ch * 32:(ch + 1) * 32],
                             start=(ch == 0), stop=(ch == 1))
        gf = sb.tile([64, 32], f32, tag="gf")
        nc.sync.dma_start(out=gf[:], in_=inp["gfr"][:])
        b2f = sb.tile([64, 32], f32, tag="b2f")
        nc.sync.dma_start(out=b2f[:], in_=inp["b2fr"][:])
        zf = sb.tile([64, 32], f32, tag="zf")
        nc.vector.tensor_tensor(out=zf[:], in0=z1p[:], in1=gf[:], op=AO.mult)
        nc.vector.tensor_tensor(out=zf[:], in0=zf[:], in1=b2f[:], op=AO.add)
        zn2 = sb.tile([64, 32], f32, tag="zn2")
        nc.vector.tensor_scalar(out=zn2[:], in0=zf[:], scalar1=0.0,
                                scalar2=None, op0=AO.min)
        nc.scalar.activation(out=zn2[:], in_=zn2[:], func=AF.Exp)
        rl2 = sb.tile([64, 32], f32, tag="rl2")
        nc.scalar.activation(out=rl2[:], in_=zf[:], func=AF.Relu)
        nc.vector.scalar_tensor_tensor(out=zf[:], in0=zn2[:], scalar=-1.0,
                                       in1=rl2[:], op0=AO.add, op1=AO.add)
        ztp = pst.tile([32, 64], f32, tag="ps3")
        nc.tensor.transpose(ztp[:], zf[:], identf[0:64, 0:64])
        zts = sb.tile([32, 64], f32, tag="zts")
        nc.vector.tensor_copy(out=zts[:], in_=ztp[:])
        wf2 = sb.tile([32, 2], f32, tag="wf2")
        nc.sync.dma_start(out=wf2[:], in_=inp["Wf2"][:])
        z2p = pst.tile([64, 2], f32, tag="ps3")
        nc.tensor.matmul(z2p[:], zts[:], wf2[:], start=True, stop=True)
        bf2 = sb.tile([64, 2], f32, tag="bf2")
        nc.sync.dma_start(out=bf2[:], in_=inp["bf2r"][:])
        z2 = sb.tile([64, 2], f32, tag="z2")
        nc.vector.tensor_tensor(out=z2[:], in0=z2p[:], in1=bf2[:], op=AO.add)
        mrow = sb.tile([64, 1], f32, tag="mrow")
        nc.vector.tensor_reduce(out=mrow[:], in_=z2[:], axis=AX.X, op=AO.max)
        nc.vector.tensor_scalar(out=z2[:], in0=z2[:], scalar1=mrow[:],
                                scalar2=None, op0=AO.subtract)
        ez = sb.tile([64, 2], f32, tag="ez")
        nc.scalar.activation(out=ez[:], in_=z2[:], func=AF.Exp)
        ssum = sb.tile([64, 1], f32, tag="ssum")
        nc.vector.tensor_reduce(out=ssum[:], in_=ez[:], axis=AX.X, op=AO.add)
        nc.scalar.activation(out=ssum[:], in_=ssum[:], func=AF.Ln)
        nc.vector.tensor_scalar(out=z2[:], in0=z2[:], scalar1=ssum[:],
                                scalar2=None, op0=AO.subtract)
        nc.sync.dma_start(out=out_d[:, :], in_=z2[:])

    nc.compile()
    return nc


# ---------------------------------------------------------------- entry point
def kernel(**inputs):
    in_maps, cfg = _prep(inputs)
    if cfg not in _CACHE:
        _CACHE[cfg] = _build(cfg)
    nc = _CACHE[cfg]
    from concourse.bass_utils import run_bass_kernel_spmd
    res = run_bass_kernel_spmd(nc, in_maps, list(range(D))).results
    return np.asarray(res[0]["out"], dtype=np.float32)


# revision 3
# speedup vs baseline: 1.1336x; 1.1336x over previous
"""Trainium2 Bass kernel for nn_GAT_n2v_mean (3-layer edge-featured GAT +
mean-pool + MLP), v2.

Strategy: edges partitioned by dst owner across 8 cores (6250 nodes each,
127-node blocks + trash slot). Per-edge src features come from batched
int16-indexed dma_gathers (<=1024 idx/instr, trailing -1 skipped) out of
bf16 node-feature tables with 256B-aligned row strides. Attention-logit
source/dst terms ride the tables as extra matmul columns (W_ext); the dst
term is expanded per edge via a host-baked transposed one-hot (snsT) and
small PE matmuls. Layer 1's table is computed locally on every device (x is
replicated), so only layers 2/3 AllGather their tables. Segment softmax uses
a per-device shift (max al_s + max al_d over produced rows) which is exact
for dst-owned segments - no AllReduce needed. Scatter-adds are one-hot
matmuls accumulating in PSUM; everything data-sized is bf16, accumulation
fp32.
"""

import numpy as np
import ml_dtypes

BFL = ml_dtypes.bfloat16

# ---------------------------------------------------------------- host config
N, E, G, D = 50000, 800000, 64, 8
NPD = N // D
BLK = 127
NB = (NPD + BLK - 1) // BLK       # 50
R = NB * 128                       # 6400 padded local rows
QROWS = 4 * R                      # 25600 rows per quad table slice
EPS = 1e-5
BNC = float(1.0 / np.sqrt(1.0 + EPS))
DIMS = [(32, 4, 64), (256, 4, 128), (512, 4, 64)]
HCs = [256, 512, 256]
LIVE = [264, 520, 264]             # HC + 8 live row elems
STRIDE = [384, 640, 384]           # padded table row stride (x2B % 256 == 0)
GCAP = 1024                        # max idxs per dma_gather instruction

_CACHE = {}


def _wrap_idx(ids, ncols, pad=-1):
    """int16 idx layout: value i at [16k + i%16, i//16] for k in 0..7."""
    a = np.full((128, ncols), pad, np.int16)
    n = len(ids)
    cols = (n + 15) // 16
    block = np.full(16 * cols, pad, np.int16)
    block[:n] = ids
    block = block.reshape(cols, 16).T          # [16, cols]
    for k in range(8):
        a[16 * k:16 * k + 16, :cols] = block
    return a


def _prep(inputs, simpad=True):
    x = np.asarray(inputs["x"], np.float32)
    ef = np.asarray(inputs["edge_feature"], np.float32)
    src_g = np.asarray(inputs["edge_index"][0], np.int64)
    dst_g = np.asarray(inputs["edge_index"][1], np.int64)
    batch = np.asarray(inputs["batch"], np.int64)

    CH = R // 2          # 3200 local rows per AG chunk

    def gr(n):   # global table row id (chunk-major)
        dn = n // NPD
        ln = n % NPD
        c = ln // CH
        return c * (D * CH) + dn * CH + (ln % CH)

    # ---- per-device edge layout
    per_dev = []
    max_cnt = 0
    for d in range(D):
        m = (dst_g // NPD) == d
        s, t, f = src_g[m], dst_g[m], ef[m]
        loc = t - d * NPD
        b = loc // BLK
        rel = loc % BLK
        srow = gr(s)
        q = (s % NPD) // (R // 2)
        order = np.lexsort((np.arange(len(s)), q, b))
        s, f, b, rel, srow, q = (a[order] for a in (s, f, b, rel, srow, q))
        cnts = np.zeros((NB, 2), np.int64)
        np.add.at(cnts, (b, q), 1)
        max_cnt = max(max_cnt, int(cnts.max()))
        per_dev.append((f, b, rel, srow, q, cnts))

    cA = (min(max_cnt, GCAP) + 127) // 128
    rest = max(max_cnt - GCAP, 16)
    cB = (rest + 127) // 128 if max_cnt > GCAP else 1
    T = 2 * (cA + cB)
    cfg = (T, cA, cB)

    segs = [(0, 0, cA), (0, cA, cB), (1, cA + cB, cA), (1, 2 * cA + cB, cB)]
    # (quad, chunk base, n chunks); segment A first min(cnt, cA*128) edges

    in_maps = []
    for d in range(D):
        f, b, rel, srow, q, cnts = per_dev[d]
        off = np.zeros((NB, 2), np.int64)
        # start offset of (block, quad) runs in the sorted edge arrays
        flat = np.concatenate([[0], np.cumsum(cnts.reshape(-1))])[:-1]
        off = flat.reshape(NB, 2)

        rels = np.full((NB, T * 128), 127, np.int64)
        pad = 0 if simpad else -1
        idx16 = np.full((NB, 128, T * 8), pad, np.int16)
        eatt = np.zeros((NB, 6, T * 128), np.float32)
        eart = np.zeros((NB, 128, T * 8), np.float32)

        for blk in range(NB):
            for (qq, cbase, nch) in segs:
                e0 = off[blk, qq]
                cnt = cnts[blk, qq]
                is_a = cbase in (0, cA + cB)
                if is_a:
                    k = min(cnt, cA * 128)
                    el = np.arange(e0, e0 + k)
                    ids = (srow[el] % QROWS).astype(np.int16)
                    rl = rel[el]
                    fe = f[el]
                else:
                    k0 = min(cnt, cA * 128)
                    k = cnt - k0
                    if k > 0:
                        el = np.arange(e0 + k0, e0 + cnt)
                        ids = (srow[el] % QROWS).astype(np.int16)
                        rl = rel[el]
                        fe = f[el]
                    else:
                        k = 16
                        ids = np.zeros(16, np.int16)
                        rl = np.full(16, 127, np.int64)
                        fe = np.zeros((16, 6), np.float32)
                assert k <= nch * 128, (d, blk, qq, k, nch)
                ii = np.arange(k)
                sl = (cbase + ii // 128) * 128 + ii % 128
                rels[blk, sl] = rl
                eatt[blk, :, sl] = fe
                tt = sl // 128
                pp = sl % 128
                for j in range(6):
                    eart[blk, pp, tt * 8 + j] = fe[:, j]
                eart[blk, pp, tt * 8 + 6] = 1.0
                idx16[blk, :, cbase * 8:(cbase + nch) * 8] = _wrap_idx(
                    ids, nch * 8, pad)

        # snsT[n, t*128+p] = 1{rels[t*128+p] == n}
        snsT = (rels[:, None, :] == np.arange(128)[None, :, None])
        snsT = snsT.astype(BFL)                       # [NB, 128, T*128]

        # rel per (block, slot) -> [128, T] tile layout: rel[p, t] = slot t*128+p
        relb = rels.reshape(NB, T, 128).transpose(0, 2, 1).astype(BFL)

        bcol = np.full((NB, 128), -1.0, np.float32)
        for blk in range(NB):
            lo = blk * BLK
            nreal = min(BLK, NPD - lo)
            if nreal > 0:
                bcol[blk, :nreal] = batch[d * NPD + lo: d * NPD + lo + nreal]

        xT_full = np.zeros((32, D * R), np.float32)
        CH = R // 2
        for c in range(2):
            for dd in range(D):
                lo = c * CH
                n_real = min(CH, NPD - lo)
                if n_real > 0:
                    xT_full[:, c * D * CH + dd * CH:
                            c * D * CH + dd * CH + n_real] = \
                        x[dd * NPD + lo: dd * NPD + lo + n_real].T
        xT_loc = np.zeros((32, R), np.float32)
        xT_loc[:, :NPD] = x[d * NPD:(d + 1) * NPD].T

        im = {
            "relb": relb.reshape(NB * 128, T).copy(),
            "gidx": idx16.reshape(NB * 128, T * 8).copy(),
            "snsT": snsT.reshape(NB * 128, T * 128).copy(),
            "eaT": np.ascontiguousarray(eatt.transpose(1, 0, 2)
                                        .reshape(6, NB * T * 128)).astype(BFL),
            "eart": eart.reshape(NB * 128, T * 8).astype(BFL),
            "bcol": bcol.reshape(NB * 128, 1).astype(BFL),
            "xT_full": xT_full.astype(BFL),
            "xT_loc": xT_loc.astype(BFL),
            "iota128": np.broadcast_to(np.arange(128, dtype=np.float32),
                                       (128, 128)).astype(BFL).copy(),
            "iota64": np.broadcast_to(np.arange(64, dtype=np.float32),
                                      (128, 64)).astype(BFL).copy(),
            "identb": np.eye(128, dtype=np.float32).astype(BFL),
            "identf": np.eye(128, dtype=np.float32),
            "onescol": np.ones((128, 1), np.float32).astype(BFL),
        }

        Aecat = np.zeros((6, 12), np.float32)
        for li, (fin, H, C) in enumerate(DIMS, 1):
            HC = H * C
            W = np.asarray(inputs[f"W{li}"], np.float32)
            a_s = np.asarray(inputs[f"as{li}"], np.float32)
            a_d = np.asarray(inputs[f"ad{li}"], np.float32)
            a_e = np.asarray(inputs[f"ae{li}"], np.float32)
            We = np.asarray(inputs[f"We{li}"], np.float32)
            Was = np.stack([W[:, h * C:(h + 1) * C] @ a_s[h]
                            for h in range(H)], axis=1)
            Wad = np.stack([W[:, h * C:(h + 1) * C] @ a_d[h]
                            for h in range(H)], axis=1)
            Wext = np.concatenate([W, Was, Wad], axis=1)   # [fin, HC+8]
            im[f"Wext{li}"] = Wext.astype(BFL)
            Aecat[:, (li - 1) * 4: li * 4] = np.stack(
                [We[:, h * C:(h + 1) * C] @ a_e[h] for h in range(H)], axis=1)
            g = np.asarray(inputs[f"g{li}"], np.float32)
            bb = np.asarray(inputs[f"bb{li}"], np.float32)
            bl = np.asarray(inputs[f"b{li}"], np.float32)
            ghat = g * BNC
            b2 = ghat * bl + bb
            im[f"ghat{li}"] = np.broadcast_to(ghat, (128, HC)).astype(BFL).copy()
            im[f"b2{li}"] = np.broadcast_to(b2, (128, HC)).astype(BFL).copy()
        im["Aecat"] = Aecat.astype(BFL)
        im["Wf1"] = np.asarray(inputs["Wf1"], np.float32)
        im["Wf2"] = np.asarray(inputs["Wf2"], np.float32)
        gf = np.asarray(inputs["gf"], np.float32) * BNC
        b2f = gf * np.asarray(inputs["bf1"], np.float32) + \
            np.asarray(inputs["bbf"], np.float32)
        im["gfr"] = np.broadcast_to(gf, (64, 32)).copy()
        im["b2fr"] = np.broadcast_to(b2f, (64, 32)).copy()
        im["bf2r"] = np.broadcast_to(np.asarray(inputs["bf2"], np.float32),
                                     (64, 2)).copy()
        in_maps.append(im)
    return in_maps, cfg


# ---------------------------------------------------------------- device prog
def _build(cfg, unroll=False, dbg=False):
    import concourse.bass as bass
    import concourse.bacc as bacc
    import concourse.mybir as mybir
    import concourse.tile as tile
    from contextlib import ExitStack

    T, cA, cB = cfg
    f32 = mybir.dt.float32
    bf16 = mybir.dt.bfloat16
    i16 = mybir.dt.int16
    AO = mybir.AluOpType
    AF = mybir.ActivationFunctionType
    AX = mybir.AxisListType
    RG = [list(range(D))]
    segs = [(0, 0, cA), (0, cA, cB), (1, cA + cB, cA), (1, 2 * cA + cB, cB)]

    nc = bacc.Bacc(None, target_bir_lowering=False, debug=True)

    inp = {}

    def di(name, shape, dt=bf16):
        inp[name] = nc.declare_dram_parameter(name, list(shape), dt,
                                              isOutput=False)
        return inp[name]

    di("relb", (NB * 128, T))
    di("gidx", (NB * 128, T * 8), i16)
    di("snsT", (NB * 128, T * 128))
    di("eaT", (6, NB * T * 128))
    di("eart", (NB * 128, T * 8))
    di("bcol", (NB * 128, 1))
    di("xT_full", (32, D * R))
    di("xT_loc", (32, R))
    di("iota128", (128, 128)); di("iota64", (128, 64))
    di("identb", (128, 128)); di("identf", (128, 128), f32)
    di("onescol", (128, 1))
    di("Aecat", (6, 12))
    for li, (fin, H, C) in enumerate(DIMS, 1):
        HC = H * C
        di(f"Wext{li}", (fin, HC + 8))
        di(f"ghat{li}", (128, HC)); di(f"b2{li}", (128, HC))
    di("Wf1", (256, 32), f32); di("Wf2", (32, 2), f32)
    di("gfr", (64, 32), f32); di("b2fr", (64, 32), f32)
    di("bf2r", (64, 2), f32)
    out_d = nc.declare_dram_parameter("out", [64, 2], f32, isOutput=True)
    dbg_d = {}
    if dbg:
        for nm, shp in [("dxf1", (128, 384)), ("dxe2", (128, 640)),
                        ("dxe3", (128, 384)), ("dale", (128, 12 * T)),
                        ("demean", (8, 128)), ("dpool", (64, 257)),
                        ("dmh", (128, 12))]:
            dbg_d[nm] = nc.declare_dram_parameter(nm, list(shp), f32,
                                                  isOutput=True)

    # internal DRAM
    xf = [nc.dram_tensor(f"xf{l}", [D * R, STRIDE[l - 1]], bf16,
                         **({"addr_space": "Shared"} if l > 1 else {}))
          for l in (1, 2, 3)]
    xe = [nc.dram_tensor(f"xe{l}", [R, STRIDE[l - 1]], bf16) for l in (1, 2, 3)]
    ale_d = nc.dram_tensor("ale", [NB * 128, 12 * T], bf16)
    emeanT_d = nc.dram_tensor("emeanT", [8, NB * 128], bf16)
    pool_i = nc.dram_tensor("pool_i", [64, 257], f32)
    pool_o = nc.dram_tensor("pool_o", [64, 257], f32, addr_space="Shared")

    with ExitStack() as ctx:
        tc = ctx.enter_context(tile.TileContext(nc))

        def loop(n, body, K=1):
            if unroll:
                for i in range(n):
                    body(i)
            else:
                if K > 1:
                    assert n % K == 0
                    with tc.For_i(0, n // K, 1) as i:
                        for j in range(K):
                            body(i * K + j)
                else:
                    with tc.For_i(0, n, 1) as i:
                        body(i)

        consts = ctx.enter_context(tc.tile_pool(name="consts", bufs=1))
        lay = ctx.enter_context(tc.tile_pool(name="lay", bufs=1))
        glob = ctx.enter_context(tc.tile_pool(name="glob", bufs=1))
        sb = ctx.enter_context(tc.tile_pool(name="sb", bufs=3))
        sbg = ctx.enter_context(tc.tile_pool(name="sbg", bufs=3))
        sbv = ctx.enter_context(tc.tile_pool(name="sbv", bufs=4))
        psb = ctx.enter_context(tc.tile_pool(name="psb", bufs=3, space="PSUM"))
        pss = ctx.enter_context(tc.tile_pool(name="pss", bufs=2, space="PSUM"))
        pst = ctx.enter_context(tc.tile_pool(name="pst", bufs=3, space="PSUM"))

        io128 = consts.tile([128, 128], bf16)
        nc.sync.dma_start(out=io128[:], in_=inp["iota128"][:])
        io64 = consts.tile([128, 64], bf16)
        nc.sync.dma_start(out=io64[:], in_=inp["iota64"][:])
        identb = consts.tile([128, 128], bf16)
        nc.sync.dma_start(out=identb[:], in_=inp["identb"][:])
        identf = consts.tile([128, 128], f32)
        nc.sync.dma_start(out=identf[:], in_=inp["identf"][:])
        onescol = consts.tile([128, 1], bf16)
        nc.sync.dma_start(out=onescol[:], in_=inp["onescol"][:])
        onesrow = consts.tile([1, 128], f32)
        nc.any.memset(onesrow[:], 1.0)
        Aecat = consts.tile([6, 12], bf16)
        nc.sync.dma_start(out=Aecat[:], in_=inp["Aecat"][:])
        w1 = consts.tile([32, 264], bf16)
        nc.sync.dma_start(out=w1[:], in_=inp["Wext1"][:])

        mx = [glob.tile([128, 8], f32, tag=f"mx{l}", name=f"mx{l}")
              for l in (1, 2, 3)]
        for t_ in mx:
            nc.any.memset(t_[:], -3e38)
        mhat = [glob.tile([128, 4], f32, tag=f"mh{l}", name=f"mh{l}")
                for l in (1, 2, 3)]
        pool_sb = glob.tile([64, 257], f32)
        nc.any.memset(pool_sb[:], 0.0)

        # ---------------- phase 1: xf1 / xe1 local projection ----------------
        def proj_group(g, nt, src, dst, track_mx):
            st = g * nt * 128
            ht = sbv.tile([32, nt * 128], bf16, tag="pht")
            nc.sync.dma_start(out=ht[:], in_=src[:, bass.ds(st, nt * 128)])
            for j in range(nt):
                pxs = pst.tile([128, 264], f32, tag="ps3")
                nc.tensor.matmul(pxs[:], ht[:, j * 128:(j + 1) * 128], w1[:],
                                 start=True, stop=True)
                xs = sbv.tile([128, 264], bf16, tag="pxs")
                nc.scalar.activation(out=xs[:], in_=pxs[:], func=AF.Copy)
                if track_mx:
                    nc.vector.tensor_tensor(out=mx[0][:], in0=mx[0][:],
                                            in1=pxs[:, 256:264], op=AO.max)
                nc.sync.dma_start(
                    out=dst[bass.ds(st + j * 128, 128), 0:264], in_=xs[:])

        # ---------------- phase 2: precompute emean + al_e ----------------
        def pre_body(i):
            st = i * 128
            rel = sb.tile([128, T], bf16, tag="prel")
            nc.sync.dma_start(out=rel[:], in_=inp["relb"][bass.ds(st, 128), :])
            ear = sb.tile([128, T * 8], bf16, tag="pear")
            nc.sync.dma_start(out=ear[:], in_=inp["eart"][bass.ds(st, 128), :])
            eatt = sb.tile([6, T * 128], bf16, tag="peatt")
            nc.scalar.dma_start(out=eatt[:],
                                in_=inp["eaT"][:, bass.ts(i, T * 128)])
            sall = sbg.tile([128, T * 128], bf16, tag="sall")
            nc.vector.tensor_tensor(
                out=sall[:].rearrange("p (t n) -> p t n", t=T),
                in0=rel[:].unsqueeze(2).to_broadcast([128, T, 128]),
                in1=io128[:].unsqueeze(1).to_broadcast([128, T, 128]),
                op=AO.is_equal)
            acc = pss.tile([128, 8], f32, tag="sm")
            alp = psb.tile([128, 12 * T], f32, tag="big")
            for t in range(T):
                nc.tensor.matmul(acc[:], sall[:, t * 128:(t + 1) * 128],
                                 ear[:, t * 8:(t + 1) * 8],
                                 start=(t == 0), stop=(t == T - 1))
                nc.tensor.matmul(alp[:, t * 12:(t + 1) * 12],
                                 eatt[:, t * 128:(t + 1) * 128],
                                 Aecat[:], start=True, stop=True)
            ale3 = sb.tile([128, 12 * T], bf16, tag="ale3")
            nc.vector.tensor_copy(
                out=ale3[:].rearrange("p (l t h) -> p l t h", l=3, t=T),
                in_=alp[:].rearrange("p (t l h) -> p l t h", t=T, l=3))
            nc.scalar.dma_start(out=ale_d[bass.ds(st, 128), :], in_=ale3[:])
            degc = sb.tile([128, 1], f32, tag="degc")
            nc.vector.tensor_scalar(out=degc[:], in0=acc[:, 6:7], scalar1=1.0,
                                    scalar2=None, op0=AO.max)
            nc.vector.reciprocal(out=degc[:], in_=degc[:])
            em = sb.tile([128, 8], f32, tag="em")
            nc.vector.tensor_scalar(out=em[:], in0=acc[:], scalar1=degc[:],
                                    scalar2=None, op0=AO.mult)
            emt_ps = pst.tile([8, 128], f32, tag="ps3")
            nc.tensor.transpose(emt_ps[:], em[:], identf[:])
            emt = sb.tile([8, 128], bf16, tag="emts")
            nc.vector.tensor_copy(out=emt[:], in_=emt_ps[:])
            nc.sync.dma_start(out=emeanT_d[:, bass.ds(st, 128)], in_=emt[:])

        def merged_body(i):
            pre_body(i)
            proj_group(i, 8, inp["xT_full"], xf[0], True)
            proj_group(i, 1, inp["xT_loc"], xe[0], False)

        loop(NB, merged_body)

        # mhat1 from mx[0]
        def build_mhat(l):
            mx_ps = pst.tile([8, 128], f32, tag="ps3")
            nc.tensor.transpose(mx_ps[:], mx[l - 1][:], identf[:])
            mx_sb = sb.tile([8, 128], f32, tag="mxsb")
            nc.vector.tensor_copy(out=mx_sb[:], in_=mx_ps[:])
            t32 = sb.tile([32, 32], f32, tag="t32")
            nc.any.memset(t32[:], -3e38)
            nc.vector.tensor_reduce(out=t32[0:8, 0:1], in_=mx_sb[:],
                                    axis=AX.X, op=AO.max)
            v32 = sb.tile([32, 32], f32, tag="v32")
            nc.vector.transpose(out=v32[:], in_=t32[:])
            mh1 = sb.tile([1, 4], f32, tag="mh1")
            nc.vector.tensor_tensor(out=mh1[:], in0=v32[0:1, 0:4],
                                    in1=v32[0:1, 4:8], op=AO.add)
            mh_ps = pst.tile([128, 4], f32, tag="ps3")
            nc.tensor.matmul(mh_ps[:], onesrow[:], mh1[:], start=True,
                             stop=True)
            nc.vector.tensor_copy(out=mhat[l - 1][:], in_=mh_ps[:])

        build_mhat(1)

        # ---------------- phase 3: attention layers ----------------
        for li, (fin, H, C) in enumerate(DIMS, 1):
            HC = H * C
            S = STRIDE[li - 1]
            LV = LIVE[li - 1]
            xf_l = xf[li - 1]
            if li < 3:
                HCn = HCs[li]
                Sn = STRIDE[li]
                nk = HC // 128
                wsb = lay.tile([128, nk * (HCn + 8)], bf16, tag="wsb")
                for k in range(nk):
                    nc.sync.dma_start(
                        out=wsb[:, k * (HCn + 8):(k + 1) * (HCn + 8)],
                        in_=inp[f"Wext{li + 1}"][k * 128:(k + 1) * 128, :])
            ghat = lay.tile([128, HC], bf16, tag="ghat")
            nc.sync.dma_start(out=ghat[:], in_=inp[f"ghat{li}"][:])
            b2 = lay.tile([128, HC], bf16, tag="b2")
            nc.sync.dma_start(out=b2[:], in_=inp[f"b2{li}"][:])

            def att_body(i, li=li, HC=HC, S=S, LV=LV, xf_l=xf_l, H=H, C=C):
                st = i * 128
                stb = i * BLK
                rel = sb.tile([128, T], bf16, tag="arel")
                nc.sync.dma_start(out=rel[:],
                                  in_=inp["relb"][bass.ds(st, 128), :])
                gix = sb.tile([128, T * 8], i16, tag="gix")
                nc.sync.dma_start(out=gix[:],
                                  in_=inp["gidx"][bass.ds(st, 128), :])
                snsT = sbg.tile([128, T * 128], bf16, tag="snsT")
                nc.scalar.dma_start(out=snsT[:],
                                    in_=inp["snsT"][bass.ds(st, 128), :])
                xsl = sb.tile([128, LV], bf16, tag="xsl")
                nc.sync.dma_start(out=xsl[:],
                                  in_=xe[li - 1][bass.ds(stb, 128), 0:LV])
                alet = sb.tile([128, 4 * T], bf16, tag="alet")
                nc.sync.dma_start(
                    out=alet[:],
                    in_=ale_d[bass.ds(st, 128),
                              (li - 1) * 4 * T: li * 4 * T])
                emt = sb.tile([8, 128], bf16, tag="emt")
                nc.scalar.dma_start(out=emt[:],
                                    in_=emeanT_d[:, bass.ds(st, 128)])
                gat = sbg.tile([128, T * S], bf16, tag="gat")
                for (qq, cbase, nch) in segs:
                    nc.gpsimd.dma_gather(
                        gat[:, cbase * S:(cbase + nch) * S].rearrange(
                            "p (c w) -> p c w", c=nch),
                        xf_l[qq * QROWS:(qq + 1) * QROWS, :],
                        gix[:, cbase * 8:(cbase + nch) * 8],
                        nch * 128, nch * 128, S, elem_step=S)
                sall = sbg.tile([128, T * 128], bf16, tag="sall")
                nc.vector.tensor_tensor(
                    out=sall[:].rearrange("p (t n) -> p t n", t=T),
                    in0=rel[:].unsqueeze(2).to_broadcast([128, T, 128]),
                    in1=io128[:].unsqueeze(1).to_broadcast([128, T, 128]),
                    op=AO.is_equal)
                atp = pss.tile([128, 4 * T], f32, tag="sm")
                for t in range(T):
                    nc.tensor.matmul(atp[:, t * 4:(t + 1) * 4],
                                     snsT[:, t * 128:(t + 1) * 128],
                                     xsl[:, HC + 4:HC + 8],
                                     start=True, stop=True)
                wall = sb.tile([128, 4 * T], f32, tag="wall")
                nc.vector.tensor_tensor(
                    out=wall[:],
                    in0=gat[:].rearrange("p (t s) -> p t s", t=T)[
                        :, :, HC:HC + 4],
                    in1=atp[:], op=AO.add)
                nc.vector.tensor_tensor(out=wall[:], in0=wall[:],
                                        in1=alet[:], op=AO.add)
                nc.vector.scalar_tensor_tensor(
                    out=wall[:], in0=wall[:], scalar=0.2, in1=wall[:],
                    op0=AO.mult, op1=AO.max)
                nc.vector.tensor_tensor(
                    out=wall[:].rearrange("p (t h) -> p t h", t=T),
                    in0=wall[:].rearrange("p (t h) -> p t h", t=T),
                    in1=mhat[li - 1][:].unsqueeze(1).to_broadcast([128, T, 4]),
                    op=AO.subtract)
                walle = sb.tile([128, 4 * T], bf16, tag="walle")
                nc.scalar.activation(out=walle[:], in_=wall[:], func=AF.Exp)
                nps = psb.tile([128, HC], f32, tag="big")
                dps = pss.tile([128, 4], f32, tag="sm")
                for t in range(T):
                    val = sbv.tile([128, HC], bf16, tag="val")
                    nc.vector.tensor_tensor(
                        out=val[:].rearrange("p (h c) -> p h c", h=H),
                        in0=gat[:, t * S:t * S + HC].rearrange(
                            "p (h c) -> p h c", h=H),
                        in1=walle[:, t * 4:(t + 1) * 4].unsqueeze(2)
                        .to_broadcast([128, 4, C]),
                        op=AO.mult)
                    nc.tensor.matmul(nps[:], sall[:, t * 128:(t + 1) * 128],
                                     val[:], start=(t == 0), stop=(t == T - 1))
                    nc.tensor.matmul(dps[:], sall[:, t * 128:(t + 1) * 128],
                                     walle[:, t * 4:(t + 1) * 4],
                                     start=(t == 0), stop=(t == T - 1))
                # epilogue: self-loop + normalize + BN + ELU
                aesp = pst.tile([128, 4], f32, tag="ps3")
                nc.tensor.matmul(aesp[:], emt[0:6, :],
                                 Aecat[:, (li - 1) * 4:li * 4],
                                 start=True, stop=True)
                als = sb.tile([128, 4], f32, tag="als")
                nc.vector.tensor_tensor(out=als[:], in0=xsl[:, HC:HC + 4],
                                        in1=xsl[:, HC + 4:HC + 8], op=AO.add)
                nc.vector.tensor_tensor(out=als[:], in0=als[:], in1=aesp[:],
                                        op=AO.add)
                nc.vector.scalar_tensor_tensor(
                    out=als[:], in0=als[:], scalar=0.2, in1=als[:],
                    op0=AO.mult, op1=AO.max)
                nc.vector.tensor_tensor(out=als[:], in0=als[:],
                                        in1=mhat[li - 1][:], op=AO.subtract)
                alse = sb.tile([128, 4], f32, tag="alse")
                nc.scalar.activation(out=alse[:], in_=als[:], func=AF.Exp)
                den = sb.tile([128, 4], f32, tag="den")
                nc.vector.tensor_tensor(out=den[:], in0=dps[:], in1=alse[:],
                                        op=AO.add)
                nc.vector.reciprocal(out=den[:], in_=den[:])
                hh = sb.tile([128, HC], bf16, tag="hh")
                for h in range(H):
                    hsl = hh[:, h * C:(h + 1) * C]
                    nc.vector.scalar_tensor_tensor(
                        out=hsl, in0=xsl[:, h * C:(h + 1) * C],
                        scalar=alse[:, h:h + 1],
                        in1=nps[:, h * C:(h + 1) * C],
                        op0=AO.mult, op1=AO.add)
                    nc.vector.tensor_scalar(
                        out=hsl, in0=hsl, scalar1=den[:, h:h + 1],
                        scalar2=None, op0=AO.mult)
                nc.vector.tensor_tensor(out=hh[:], in0=hh[:], in1=ghat[:],
                                        op=AO.mult)
                nc.vector.tensor_tensor(out=hh[:], in0=hh[:], in1=b2[:],
                                        op=AO.add)
                zn = sb.tile([128, HC], bf16, tag="zn")
                nc.vector.tensor_scalar(out=zn[:], in0=hh[:], scalar1=0.0,
                                        scalar2=None, op0=AO.min)
                nc.scalar.activation(out=zn[:], in_=zn[:], func=AF.Exp)
                rl = sb.tile([128, HC], bf16, tag="rl")
                nc.scalar.activation(out=rl[:], in_=hh[:], func=AF.Relu)
                nc.vector.scalar_tensor_tensor(
                    out=hh[:], in0=zn[:], scalar=-1.0, in1=rl[:],
                    op0=AO.add, op1=AO.add)
                if li < 3:
                    HCn = HCs[li]
                    nk = HC // 128
                    pxa = psb.tile([128, min(HCn, 512)], f32, tag="big")
                    pxb = pss.tile([128, 8], f32, tag="sm")
                    for k in range(nk):
                        trp = pst.tile([128, 128], bf16, tag="ps3")
                        nc.tensor.transpose(trp[:],
                                            hh[:, k * 128:(k + 1) * 128],
                                            identb[:])
                        htc = sbv.tile([128, 128], bf16, tag="htc")
                        nc.vector.tensor_copy(out=htc[:], in_=trp[:])
                        wk = wsb[:, k * (HCn + 8):(k + 1) * (HCn + 8)]
                        nc.tensor.matmul(pxa[:], htc[:], wk[:, 0:HCn],
                                         start=(k == 0), stop=(k == nk - 1))
                        nc.tensor.matmul(pxb[:], htc[:], wk[:, HCn:HCn + 8],
                                         start=(k == 0), stop=(k == nk - 1))
                    xs = sb.tile([128, HCn + 8], bf16, tag="xsout")
                    nc.scalar.activation(out=xs[:, 0:HCn], in_=pxa[:],
                                         func=AF.Copy)
                    nc.vector.tensor_copy(out=xs[:, HCn:HCn + 8], in_=pxb[:])
                    nc.vector.tensor_tensor(out=mx[li][0:127, :],
                                            in0=mx[li][0:127, :],
                                            in1=xs[0:127, HCn:HCn + 8],
                                            op=AO.max)
                    nc.sync.dma_start(
                        out=xe[li][bass.ds(stb, 128), 0:HCn + 8],
                        in_=xs[:])
                else:
                    bc = sb.tile([128, 1], bf16, tag="bc")
                    nc.sync.dma_start(out=bc[:],
                                      in_=inp["bcol"][bass.ds(st, 128), :])
                    bt = sb.tile([128, 64], bf16, tag="bt")
                    nc.vector.tensor_tensor(out=bt[:],
                                            in0=bc[:].to_broadcast([128, 64]),
                                            in1=io64[:], op=AO.is_equal)
                    pps = pst.tile([64, 257], f32, tag="ps3")
                    nc.tensor.matmul(pps[:, 0:HC], bt[:], hh[:],
                                     start=True, stop=True)
                    nc.tensor.matmul(pps[:, 256:257], bt[:], onescol[:],
                                     start=True, stop=True)
                    nc.vector.tensor_tensor(out=pool_sb[:], in0=pool_sb[:],
                                            in1=pps[:], op=AO.add)

            NB1 = 26
            loop(NB1, att_body, K=2)
            if li < 3:
                nc.gpsimd.collective_compute(
                    "AllGather", AO.bypass, replica_groups=RG,
                    ins=[xe[li][0:R // 2, :]],
                    outs=[xf[li][0:D * R // 2, :]])
            loop(NB - NB1, lambda i2: att_body(NB1 + i2), K=2)
            if li < 3:
                build_mhat(li + 1)
                nc.gpsimd.collective_compute(
                    "AllGather", AO.bypass, replica_groups=RG,
                    ins=[xe[li][R // 2:R, :]],
                    outs=[xf[li][D * R // 2:D * R, :]])

        if dbg:
            def dump(nm, src, shape, dt=bf16):
                tl = sb.tile(list(shape), dt, tag="dbgl", name="dbgtl")
                nc.sync.dma_start(out=tl[:], in_=src)
                tf = sb.tile(list(shape), f32, tag="dbgf", name="dbgtf")
                nc.vector.tensor_copy(out=tf[:], in_=tl[:])
                nc.sync.dma_start(out=dbg_d[nm][:], in_=tf[:])
            dump("dxf1", xf[0][0:128, :], (128, 384))
            dump("dxe2", xe[1][0:128, :], (128, 640))
            dump("dxe3", xe[2][0:128, :], (128, 384))
            dump("dale", ale_d[0:128, :], (128, 12 * T))
            dump("demean", emeanT_d[:, 0:128], (8, 128))
            nc.sync.dma_start(out=dbg_d["dpool"][:], in_=pool_sb[:])
            mhcat = sb.tile([128, 12], f32, tag="mhcat")
            for l_ in range(3):
                nc.vector.tensor_copy(out=mhcat[:, l_ * 4:(l_ + 1) * 4],
                                      in_=mhat[l_][:])
            nc.sync.dma_start(out=dbg_d["dmh"][:], in_=mhcat[:])

        # ---------------- final MLP ----------------
        nc.sync.dma_start(out=pool_i[:], in_=pool_sb[:])
        nc.gpsimd.collective_compute("AllReduce", AO.add, replica_groups=RG,
                                     ins=[pool_i[:]], outs=[pool_o[:]])
        pool2 = sb.tile([64, 257], f32, tag="pool2")
        nc.sync.dma_start(out=pool2[:], in_=pool_o[:])
        cnt = sb.tile([64, 1], f32, tag="cnt")
        nc.vector.tensor_scalar(out=cnt[:], in0=pool2[:, 256:257], scalar1=1.0,
                                scalar2=None, op0=AO.max)
        nc.vector.reciprocal(out=cnt[:], in_=cnt[:])
        nc.vector.tensor_scalar(out=pool2[:, 0:256], in0=pool2[:, 0:256],
                                scalar1=cnt[:], scalar2=None, op0=AO.mult)
        pts = sb.tile([128, 128], f32, tag="pts")
        for ch in range(2):
            ptp = pst.tile([128, 64], f32, tag="ps3")
            nc.tensor.transpose(ptp[:], pool2[:, ch * 128:(ch + 1) * 128],
                                identf[0:64, 0:64])
            nc.vector.tensor_copy(out=pts[:, ch * 64:(ch + 1) * 64],
                                  in_=ptp[:])
        wf1 = sb.tile([128, 64], f32, tag="wf1")
        for ch in range(2):
            nc.sync.dma_start(out=wf1[:, ch * 32:(ch + 1) * 32],
                              in_=inp["Wf1"][ch * 128:(ch + 1) * 128, :])
        z1p = pst.tile([64, 32], f32, tag="ps3")
        for ch in range(2):
            nc.tensor.matmul(z1p[:], pts[:, ch * 64:(ch + 1) * 64],
                             wf1[:, ch * 32:(ch + 1) * 32],
                             start=(ch == 0), stop=(ch == 1))
        gf = sb.tile([64, 32], f32, tag="gf")
        nc.sync.dma_start(out=gf[:], in_=inp["gfr"][:])
        b2f = sb.tile([64, 32], f32, tag="b2f")
        nc.sync.dma_start(out=b2f[:], in_=inp["b2fr"][:])
        zf = sb.tile([64, 32], f32, tag="zf")
        nc.vector.tensor_tensor(out=zf[:], in0=z1p[:], in1=gf[:], op=AO.mult)
        nc.vector.tensor_tensor(out=zf[:], in0=zf[:], in1=b2f[:], op=AO.add)
        zn2 = sb.tile([64, 32], f32, tag="zn2")
        nc.vector.tensor_scalar(out=zn2[:], in0=zf[:], scalar1=0.0,
                                scalar2=None, op0=AO.min)
        nc.scalar.activation(out=zn2[:], in_=zn2[:], func=AF.Exp)
        rl2 = sb.tile([64, 32], f32, tag="rl2")
        nc.scalar.activation(out=rl2[:], in_=zf[:], func=AF.Relu)
        nc.vector.scalar_tensor_tensor(out=zf[:], in0=zn2[:], scalar=-1.0,
                                       in1=rl2[:], op0=AO.add, op1=AO.add)
        ztp = pst.tile([32, 64], f32, tag="ps3")
        nc.tensor.transpose(ztp[:], zf[:], identf[0:64, 0:64])
        zts = sb.tile([32, 64], f32, tag="zts")
        nc.vector.tensor_copy(out=zts[:], in_=ztp[:])
        wf2 = sb.tile([32, 2], f32, tag="wf2")
        nc.sync.dma_start(out=wf2[:], in_=inp["Wf2"][:])
        z2p = pst.tile([64, 2], f32, tag="ps3")
        nc.tensor.matmul(z2p[:], zts[:], wf2[:], start=True, stop=True)
        bf2 = sb.tile([64, 2], f32, tag="bf2")
        nc.sync.dma_start(out=bf2[:], in_=inp["bf2r"][:])
        z2 = sb.tile([64, 2], f32, tag="z2")
        nc.vector.tensor_tensor(out=z2[:], in0=z2p[:], in1=bf2[:], op=AO.add)
        mrow = sb.tile([64, 1], f32, tag="mrow")
        nc.vector.tensor_reduce(out=mrow[:], in_=z2[:], axis=AX.X, op=AO.max)
        nc.vector.tensor_scalar(out=z2[:], in0=z2[:], scalar1=mrow[:],
                                scalar2=None, op0=AO.subtract)
        ez = sb.tile([64, 2], f32, tag="ez")
        nc.scalar.activation(out=ez[:], in_=z2[:], func=AF.Exp)
        ssum = sb.tile([64, 1], f32, tag="ssum")
        nc.vector.tensor_reduce(out=ssum[:], in_=ez[:], axis=AX.X, op=AO.add)
        nc.scalar.activation(out=ssum[:], in_=ssum[:], func=AF.Ln)
        nc.vector.tensor_scalar(out=z2[:], in0=z2[:], scalar1=ssum[:],
                                scalar2=None, op0=AO.subtract)
        nc.sync.dma_start(out=out_d[:, :], in_=z2[:])

    nc.compile()
    return nc


# ---------------------------------------------------------------- entry point
def kernel(**inputs):
    in_maps, cfg = _prep(inputs)
    if cfg not in _CACHE:
        _CACHE[cfg] = _build(cfg)
    nc = _CACHE[cfg]
    from concourse.bass_utils import run_bass_kernel_spmd
    res = run_bass_kernel_spmd(nc, in_maps, list(range(D))).results
    return np.asarray(res[0]["out"], dtype=np.float32)
